# revision 16
# baseline (speedup 1.0000x reference)
"""Trainium2 Bass kernel for nn_MemristorCNN (embedding_lookup, 8 cores).

Strategy (per sharding hint):
- Host gathers the codebook weight W1 = values[w_idx1] and ships the
  *gathered weight* in bf16, column-sharded over in_features (12544
  features = 4 conv2 output channels per core), pre-transposed to
  [12544, 512] so the fc1 moving operand streams as contiguous
  [128, 512] tiles; PSUM accumulation stays fp32.
- Conv stack runs data-parallel (4 images per core); conv1 packs
  (half-image, dy) into K with 3 dx-shift PSUM passes; conv2 packs
  (image, channel, dx-pair) into K=128 with 6 tap passes over a
  twice-replicated (dx-shifted) input.
- AllToAll redistributes conv output h from batch-sharded to
  feature-sharded; PE transposes h to feature-major; fc1 partial
  matmul streams the weight tiles; ReduceScatter sums partials so
  core c ends with images [4c, 4c+4); relu + fc2 finish on device and
  the host concatenates the per-core [4, 4] outputs.
"""

import sys

import numpy as np
import ml_dtypes

BF16NP = ml_dtypes.bfloat16

for _p in ("/opt/trn_rl_repo",):
    if _p not in sys.path:
        sys.path.insert(0, _p)

import concourse.bacc as bacc
import concourse.bass as bass  # noqa: F401
import concourse.tile as tile
from concourse import mybir
from concourse.bass_utils import run_bass_kernel_spmd

F32 = mybir.dt.float32
BF16 = mybir.dt.bfloat16
RELU = mybir.ActivationFunctionType.Relu
COPY = mybir.ActivationFunctionType.Copy

N_CORES = 8
B = 32
IMG = 224
C1, C2 = 16, 32
PH, PW = 112, 112
HH, HW = 56, 56
FEAT = C2 * HH * HW          # 100352
FSH = FEAT // N_CORES        # 12544
NK = FSH // 128              # 98
H1 = 512
NOUT = 4

_CACHE = {}


def _build_program(w_bufs: int, stop_after: str = 'full'):
    nc = bacc.Bacc("TRN2", target_bir_lowering=False, debug=False,
                   num_devices=N_CORES)
    _emit(nc, w_bufs, stop_after)
    nc.compile()
    return nc


def _emit(nc, w_bufs: int, stop_after: str):
    # ---- kernel I/O ----
    x9_t = nc.dram_tensor("x9", [72, PH, 232], BF16, kind="ExternalInput")
    s1_t = nc.dram_tensor("s1", [72, 128], BF16, kind="ExternalInput")
    s2_t = nc.dram_tensor("s2", [6, 128, 128], BF16, kind="ExternalInput")
    w1t_t = nc.dram_tensor("w1t", [FSH, H1], BF16, kind="ExternalInput")
    b1t_t = nc.dram_tensor("b1t", [128, 4, 4], F32, kind="ExternalInput")
    w2t_t = nc.dram_tensor("w2t", [H1, NOUT], F32, kind="ExternalInput")
    b2t_t = nc.dram_tensor("b2t", [4, 4], F32, kind="ExternalInput")
    cb1_t = nc.dram_tensor("cb1", [128, 1], F32, kind="ExternalInput")
    cb2_t = nc.dram_tensor("cb2", [128, 1], F32, kind="ExternalInput")
    ident_t = nc.dram_tensor("ident", [32, 32], BF16, kind="ExternalInput")
    out_t = nc.dram_tensor("out", [4, NOUT], F32, kind="ExternalOutput")

    # ---- internal DRAM (collective bounce buffers) ----
    a2a_in = nc.dram_tensor("a2a_in", [C2, 4, HH * HW], BF16)
    a2a_out = nc.dram_tensor("a2a_out", [N_CORES, 4, 4, HH * HW], BF16)
    rs_in = nc.dram_tensor("rs_in", [B, H1], F32)
    rs_out = nc.dram_tensor("rs_out", [4, H1], F32)

    groups = [list(range(N_CORES))]

    with tile.TileContext(nc) as tc:
        with (
            tc.tile_pool(name="wpool", bufs=w_bufs) as wpool,
            tc.tile_pool(name="const", bufs=1) as cpool,
            tc.tile_pool(name="ps", bufs=4, space="PSUM") as pspool,
            tc.tile_pool(name="work", bufs=2) as wkpool,
            tc.tile_pool(name="persist", bufs=1) as pers,
        ):
            # -------- latency-critical loads first (DMA queue order) ------
            # conv1 input: partition (dy*3+dx)*8 + h holds
            # x_pad[img(h), y0(h)+dy+y, dx+c]; row-quarters double-buffered.
            x9_tiles = []
            for q in range(4):
                x9q = wkpool.tile([72, 28, 232], BF16, tag="x9")
                nc.sync.dma_start(out=x9q[:, :, :],
                                  in_=x9_t[:, 28 * q:28 * q + 28, :])
                x9_tiles.append(x9q)

            s1_sb = cpool.tile([72, 128], BF16, tag="s1")
            nc.sync.dma_start(out=s1_sb[:, :], in_=s1_t[:, :])
            s2_sb = cpool.tile([128, 6, 128], BF16, tag="s2")
            nc.sync.dma_start(out=s2_sb[:, :, :],
                              in_=s2_t[:, :, :].rearrange("t p m -> p t m"))
            b1t_sb = cpool.tile([128, 4, 4], F32, tag="b1t")
            nc.sync.dma_start(out=b1t_sb[:, :, :], in_=b1t_t[:, :, :])
            w2t_sb = cpool.tile([128, 4, 4], F32, tag="w2t")
            nc.sync.dma_start(out=w2t_sb[:, :, :],
                              in_=w2t_t[:, :].rearrange("(k p) o -> p k o", p=128))
            b2t_sb = cpool.tile([4, 4], F32, tag="b2t")
            nc.sync.dma_start(out=b2t_sb[:, :], in_=b2t_t[:, :])
            cb1_sb = cpool.tile([128, 1], F32, tag="cb1")
            nc.sync.dma_start(out=cb1_sb[:, :], in_=cb1_t[:, :])
            cb2_sb = cpool.tile([128, 1], F32, tag="cb2")
            nc.sync.dma_start(out=cb2_sb[:, :], in_=cb2_t[:, :])
            ident_sb = cpool.tile([32, 32], BF16, tag="ident")
            nc.sync.dma_start(out=ident_sb[:, :], in_=ident_t[:, :])

            # conv2 input buffer: partition e*64 + img*16 + ch holds the
            # padded channel image, dx-shifted by e.  Cleared early so the
            # repack DMAs can land as soon as pool1 rows exist.
            c2in = pers.tile([128, 114, 116], BF16, tag="bigC")
            nc.gpsimd.memset(c2in[:, :, :].rearrange("p a b -> p (a b)"), 0.0)

            # -------- fc1 weight stream (fills SBUF buffer from t=0) ------
            wts = []
            for k in range(NK):
                wt = wpool.tile([128, H1], BF16, tag="w")
                nc.scalar.dma_start(out=wt[:, :],
                                    in_=w1t_t[128 * k:128 * k + 128, :])
                wts.append(wt)

            # ---------------- conv1 + pool1 + relu ----------------
            # out partition m = h*16 + oc = img*32 + half*16 + oc
            pool1_a = pers.tile([128, 28, PW], BF16, tag="bigB1")
            pool1_b = pers.tile([128, 28, PW], BF16, tag="bigB2")
            pool1_parts = [pool1_a, pool1_b]
            for T in range(28):            # 2 pooled rows per psum tile
                ps = pspool.tile([128, 2, 512], F32, tag="ps")
                for g in range(2):
                    yp = T * 2 + g         # pooled row within half
                    q, ypl = yp // 14, yp % 14
                    nc.tensor.matmul(
                        ps[:, g, 0:448],
                        lhsT=s1_sb[:, :],
                        rhs=x9_tiles[q][:, 2 * ypl:2 * ypl + 2, :224],
                        start=True, stop=True)
                v = ps[:, :, 0:448].rearrange("p g (r x w) -> p g r x w",
                                              r=2, w=2)
                c1 = wkpool.tile([128, 2, 2, 112], F32, tag="mc")
                nc.scalar.activation(c1[:, :, :, :], v[:, :, :, :, 1], COPY)
                m1 = wkpool.tile([128, 2, 2, 112], F32, tag="mx")
                nc.vector.tensor_max(m1[:, :, :, :], v[:, :, :, :, 0],
                                     c1[:, :, :, :])
                m2 = wkpool.tile([128, 2, 112], F32, tag="mxb")
                nc.vector.tensor_max(m2[:, :, :], m1[:, :, 0, :],
                                     m1[:, :, 1, :])
                half_t, row_t = divmod(2 * T, 28)
                nc.scalar.activation(
                    pool1_parts[half_t][:, row_t:row_t + 2, :],
                    m2[:, :, :], RELU, bias=cb1_sb[:, :])

            if stop_after == "conv1":
                dbg = wkpool.tile([4, NOUT], F32, tag="outsb")
                nc.vector.tensor_copy(dbg[:, :], pool1_a[0:4, 0, 0:4])
                nc.sync.dma_start(out=out_t[:, :], in_=dbg[:, :])
                return

            # -------- repack pool1 -> conv2 input (padded, merged halves,
            # two dx-shifted copies), in two row chunks for overlap --------
            for chunk in range(2):
                r0 = 28 * chunk
                for img in range(4):
                    for half in range(2):
                        srcp = 32 * img + 16 * half
                        for e in range(2):
                            dst = 64 * e + 16 * img
                            nc.sync.dma_start(
                                out=c2in[dst:dst + 16,
                                         56 * half + 1 + r0:
                                         56 * half + 29 + r0,
                                         1 - e:113 - e],
                                in_=pool1_parts[chunk][srcp:srcp + 16, :, :])

            # ---------------- conv2 + pool2 + relu ----------------
            # out partition m = img*32 + oc; 6 passes t=(dy, grp):
            # partition block e supplies tap dx = 2*grp + e.
            h_sb = pers.tile([128, 7, 4, 2, 56], BF16, tag="bigD")
            for T in range(14):            # 8 conv rows / 4 pooled rows
                ps = pspool.tile([128, 2, 512], F32, tag="ps")
                for sub in range(2):
                    y0 = 8 * T + 4 * sub
                    for t in range(6):
                        dy, grp = t // 2, t % 2
                        nc.tensor.matmul(
                            ps[:, sub, 0:448],
                            lhsT=s2_sb[:, t, :],
                            rhs=c2in[:, y0 + dy:y0 + dy + 4,
                                     2 * grp:2 * grp + 112],
                            start=(t == 0), stop=(t == 5))
                v = ps[:, :, 0:448].rearrange("p s (r x w) -> p s r x w",
                                              r=4, w=2)
                c1 = wkpool.tile([128, 2, 4, 56], F32, tag="mc")
                nc.scalar.activation(c1[:, :, :, :], v[:, :, :, :, 1], COPY)
                m1 = wkpool.tile([128, 2, 4, 56], F32, tag="mx")
                nc.vector.tensor_max(m1[:, :, :, :], v[:, :, :, :, 0],
                                     c1[:, :, :, :])
                v2 = m1[:, :, :, :].rearrange("p s (rp w) x -> p s rp w x",
                                              w=2)
                m2 = wkpool.tile([128, 2, 2, 56], F32, tag="mxb")
                nc.vector.tensor_max(m2[:, :, :, :], v2[:, :, :, 0, :],
                                     v2[:, :, :, 1, :])
                # pooled rows 4T..4T+4 -> h_sb[T//2, 2*(T%2) + (0..1), ...]
                nc.scalar.activation(
                    h_sb[:, T // 2, 2 * (T % 2):2 * (T % 2) + 2, :, :],
                    m2[:, :, :, :], RELU, bias=cb2_sb[:, :])

            if stop_after == "conv2":
                dbg = wkpool.tile([4, NOUT], F32, tag="outsb")
                nc.vector.tensor_copy(dbg[:, :], h_sb[0:4, 0, 0, 0, 0:4])
                nc.sync.dma_start(out=out_t[:, :], in_=dbg[:, :])
                return

            # -------- AllToAll: batch-shard -> feature-shard --------
            for img in range(4):
                nc.sync.dma_start(
                    out=a2a_in[:, img, :],
                    in_=h_sb[32 * img:32 * img + 32, :, :, :, :].rearrange(
                        "p a b c d -> p (a b c d)"))
            nc.gpsimd.collective_compute(
                "AllToAll", mybir.AluOpType.bypass, replica_groups=groups,
                ins=[a2a_in[:, :, :]], outs=[a2a_out[:, :, :, :]])

            # h_all partition img (0..31) holds all 12544 local features
            h_all = pers.tile([32, FSH], BF16, tag="bigBall")
            for i in range(N_CORES):
                nc.sync.dma_start(
                    out=h_all[4 * i:4 * i + 4, :].rearrange(
                        "p (o s) -> p o s", o=4),
                    in_=a2a_out[i, :, :, :].rearrange("o i s -> i o s"))

            if stop_after == "a2a":
                dbg = wkpool.tile([4, NOUT], F32, tag="outsb")
                nc.vector.tensor_copy(dbg[:, :], h_all[0:4, 0:4])
                nc.sync.dma_start(out=out_t[:, :], in_=dbg[:, :])
                return

            # -------- transpose h_all -> hT (feature-major) --------
            hT = pers.tile([128, NK, 32], BF16, tag="bigD")
            tp_a = pspool.tile([128, 64, 32], BF16, tag="ps")
            tp_b = pspool.tile([128, 64, 32], BF16, tag="ps")
            tp_tiles = [tp_a, tp_b]
            for k in range(NK):
                tp = tp_tiles[k // 64]
                nc.tensor.transpose(
                    tp[:, k % 64, :],
                    h_all[0:32, 128 * k:128 * k + 128],
                    ident_sb[0:32, :])
            nc.vector.tensor_copy(hT[:, 0:64, :], tp_tiles[0][:, :, :])
            nc.vector.tensor_copy(hT[:, 64:NK, :],
                                  tp_tiles[1][:, 0:NK - 64, :])

            if stop_after == "transpose":
                dbg = wkpool.tile([4, NOUT], F32, tag="outsb")
                nc.vector.tensor_copy(dbg[:, :], hT[0:4, 0, 0:4])
                nc.sync.dma_start(out=out_t[:, :], in_=dbg[:, :])
                return

            # ---------------- fc1 partial ----------------
            fc1_ps = pspool.tile([32, H1], F32, tag="ps")
            for k in range(NK):
                nc.tensor.matmul(fc1_ps[:, :], lhsT=hT[:, k, :],
                                 rhs=wts[k][:, :],
                                 start=(k == 0), stop=(k == NK - 1))
            fc1_sb = wkpool.tile([B, H1], F32, tag="fc1")
            nc.vector.tensor_copy(fc1_sb[:, :], fc1_ps[:, :])
            nc.sync.dma_start(out=rs_in[:, :], in_=fc1_sb[:, :])

            if stop_after == "fc1":
                nc.sync.dma_start(out=out_t[:, :], in_=fc1_sb[0:4, 0:4])
                return

            # -------- ReduceScatter + bias + relu + fc2 --------
            nc.gpsimd.collective_compute(
                "ReduceScatter", mybir.AluOpType.add, replica_groups=groups,
                ins=[rs_in[:, :]], outs=[rs_out[:, :]])

            h2t = wkpool.tile([128, 4, 4], F32, tag="h2t")   # [c, k, img]
            for k in range(4):
                nc.sync.dma_start(
                    out=h2t[:, k, :],
                    in_=rs_out[:, 128 * k:128 * k + 128].rearrange(
                        "i p -> p i"))
            nc.vector.tensor_add(h2t[:, :, :], h2t[:, :, :], b1t_sb[:, :, :])
            nc.scalar.activation(h2t[:, :, :], h2t[:, :, :], RELU)

            fc2_ps = pspool.tile([4, 4], F32, tag="ps")
            for k in range(4):
                nc.tensor.matmul(fc2_ps[:, :], lhsT=h2t[:, k, :],
                                 rhs=w2t_sb[:, k, :],
                                 start=(k == 0), stop=(k == 3))
            out_sb = wkpool.tile([4, NOUT], F32, tag="outsb")
            nc.vector.tensor_add(out_sb[:, :], fc2_ps[:, :], b2t_sb[:, :])
            nc.sync.dma_start(out=out_t[:, :], in_=out_sb[:, :])


def _get_program(w_bufs: int = 94):
    key = ("prog", w_bufs)
    if key not in _CACHE:
        _CACHE[key] = _build_program(w_bufs)
    return _CACHE[key]


def _host_prep(x, conv1_w, conv1_b, conv2_w, conv2_b, values, w_idx1,
               fc1_b, w_idx2, fc2_b):
    """Build per-core input maps (numpy, bf16 for PE-facing tensors)."""
    f32 = np.float32
    x = np.asarray(x, f32)
    conv1_w = np.asarray(conv1_w, f32)
    conv2_w = np.asarray(conv2_w, f32)
    values = np.asarray(values, f32)
    w_idx1 = np.asarray(w_idx1)
    w_idx2 = np.asarray(w_idx2)

    x_pad = np.zeros((B, 226, 232), f32)
    x_pad[:, 1:225, 1:225] = x[:, 0]

    # x9[c]: [72, 112, 232]; partition (dy*3+dx)*8 + h, h = 2*img_loc + half
    x9 = np.zeros((N_CORES, 72, PH, 232), f32)
    for dy in range(3):
        for dx in range(3):
            for h in range(8):
                il, half = h // 2, h % 2
                y0 = PH * half
                for c in range(N_CORES):
                    x9[c, (dy * 3 + dx) * 8 + h, :, :232 - dx] = \
                        x_pad[4 * c + il, y0 + dy:y0 + dy + PH, dx:]

    s1 = np.zeros((72, 128), f32)
    for dy in range(3):
        for dx in range(3):
            for h in range(8):
                s1[(dy * 3 + dx) * 8 + h, 16 * h:16 * h + C1] = \
                    conv1_w[:, 0, dy, dx]

    # conv2 stationaries [6, 128, 128]: pass t = dy*2 + grp;
    # partition p = e*64 + img*16 + ch supplies tap dx = 2*grp + e
    s2 = np.zeros((6, 128, 128), f32)
    for t in range(6):
        dy, grp = t // 2, t % 2
        for e in range(2):
            dx = 2 * grp + e
            if dx > 2:
                continue
            for img in range(4):
                for ch in range(C1):
                    s2[t, 64 * e + 16 * img + ch, 32 * img:32 * img + C2] = \
                        conv2_w[:, ch, dy, dx]

    w1ts = []
    for c in range(N_CORES):
        idx = w_idx1[:, FSH * c:FSH * (c + 1)]             # [512, 12544]
        w1ts.append(np.ascontiguousarray(values[idx].T).astype(BF16NP))

    b1t = np.repeat(np.asarray(fc1_b, f32).reshape(4, 128).T[:, :, None],
                    4, axis=2).copy()                       # [128, k4, img4]
    w2t = np.ascontiguousarray(values[w_idx2].T).astype(f32)  # [512, 4]
    b2t = np.broadcast_to(np.asarray(fc2_b, f32), (4, 4)).copy()

    cb1 = np.zeros((128, 1), f32)
    for h in range(8):
        cb1[16 * h:16 * h + C1, 0] = np.asarray(conv1_b, f32)
    cb2 = np.zeros((128, 1), f32)
    for img in range(4):
        cb2[32 * img:32 * img + C2, 0] = np.asarray(conv2_b, f32)

    ident = np.eye(32, dtype=f32).astype(BF16NP)

    s1 = s1.astype(BF16NP)
    s2 = s2.astype(BF16NP)
    in_maps = []
    for c in range(N_CORES):
        in_maps.append({
            "x9": np.ascontiguousarray(x9[c]).astype(BF16NP),
            "s1": s1, "s2": s2,
            "w1t": w1ts[c],
            "b1t": b1t, "w2t": w2t, "b2t": b2t,
            "cb1": cb1, "cb2": cb2, "ident": ident,
        })
    return in_maps


def kernel(x, conv1_w, conv1_b, conv2_w, conv2_b, values, w_idx1, fc1_b,
           w_idx2, fc2_b, _trace=False, _trace_kwargs=None):
    nc = _get_program()
    in_maps = _host_prep(x, conv1_w, conv1_b, conv2_w, conv2_b, values,
                         w_idx1, fc1_b, w_idx2, fc2_b)
    res = run_bass_kernel_spmd(nc, in_maps, core_ids=list(range(N_CORES)),
                               trace=_trace, **(_trace_kwargs or {}))
    out = np.zeros((B, NOUT), np.float32)
    for c in range(N_CORES):
        out[4 * c:4 * c + 4] = res.results[c]["out"]
    if _trace:
        kernel.last_result = res
    return out


if __name__ == "__main__":
    rng = np.random.default_rng(0)
    ins = {
        "x": rng.standard_normal((B, 1, IMG, IMG), dtype=np.float32),
        "conv1_w": rng.standard_normal((16, 1, 3, 3), dtype=np.float32) * 0.1,
        "conv1_b": np.zeros(16, np.float32),
        "conv2_w": rng.standard_normal((32, 16, 3, 3), dtype=np.float32) * 0.05,
        "conv2_b": np.zeros(32, np.float32),
        "values": np.sort(rng.standard_normal(4096).astype(np.float32) * 0.01),
        "w_idx1": rng.integers(0, 4096, (512, FEAT), dtype=np.int32),
        "fc1_b": np.zeros(512, np.float32),
        "w_idx2": rng.integers(0, 4096, (4, 512), dtype=np.int32),
        "fc2_b": np.zeros(4, np.float32),
    }
    out = kernel(**ins)
    print("out shape", out.shape, "sample row", out[0])
